# revision 1
# baseline (speedup 1.0000x reference)
"""Multi-head cross-attention TRN2 kernel.

N=4096, D=256, H=4, K=16. Data-parallel over 8 NeuronCores: each core owns
512 query rows, key_value + weights replicated. No collectives.

Math (per core, rows R=512):
  QT_h [16,R]   = Wq_h.T @ q^T           (q^T via DMA transpose)
  KhT_h [16,N]  = Wk_h.T @ kv^T          (kv^T via DMA transpose)
  V_aug [N,68]  = kv @ Wv_aug            (per-head 17-col groups: 16 V cols + ones col)
  per head h, per key-chunk m (128 keys):
    S^T[m,:] (PSUM) = KhT_h[:,m].T @ QT_h   -> exp(0.25*S^T) on ACT -> bf16
    heads_psum[17,R] += V_aug[m, h-group].T @ expS^T[m,:]  (row 16 = sum of exp = denom)
  headsTn[16h:,R] = heads_psum[0:16] * bcast(1/heads_psum[16])
  out[R,256] = headsTn.T @ W_o

Matmul operands must sit at base partition 0/32/64 (96 = quadrant-3 bug), and
lhsT/rhs bases must match; so heads are packed two per tile at bases {0,32}:
tile A holds heads 0,1; tile B holds heads 2,3 (both for QT and KhT).

Everything fed to the PE is bf16 (cast on host); accumulation fp32; output fp32.
Measured end-to-end absmax-relative error vs fp32 reference: ~4e-3.
"""
import numpy as np
import ml_dtypes

import concourse.bass as bass
from concourse import bacc
import concourse.mybir as mybir
import concourse.tile as tile
from concourse.bass_utils import run_bass_kernel_spmd

N, D, H, K = 4096, 256, 4, 16
NCORES = 8
R = N // NCORES          # 512 query rows per core
G = K + 1                # 17: per-head V columns + ones column
F32 = mybir.dt.float32
BF16 = mybir.dt.bfloat16
EXPF = mybir.ActivationFunctionType.Exp
BF = ml_dtypes.bfloat16

TRACE = False
LAST_RESULTS = None


def _build(repeats=1):
    nc = bacc.Bacc()
    q = nc.declare_dram_parameter("q", [R, D], BF16, isOutput=False)
    kv = nc.declare_dram_parameter("kv", [N, D], BF16, isOutput=False)
    # wqkv blob: [wq_pad(128) | wk_pad(128) | wv_aug(68)] = 324 cols per d-row;
    # wq/wk padded: head h at cols 64*(h//2)+32*(h%2) .. +16, zeros between.
    wqkv = nc.declare_dram_parameter("wqkv", [656, 128], BF16, isOutput=False)
    # wo blob: [17, 4*256], head h at cols 256h..; row 0 = zeros
    wo = nc.declare_dram_parameter("wo", [G, H * D], BF16, isOutput=False)
    out = nc.declare_dram_parameter("out", [R, D], F32, isOutput=True)

    with tile.TileContext(nc) as tc:
        with (
            tc.tile_pool(name="consts", bufs=1) as consts,
            tc.tile_pool(name="es", bufs=6) as espool,
            tc.tile_pool(name="sbops", bufs=3) as sbops,
            tc.tile_pool(name="spsum", bufs=2, space="PSUM") as spsum,
            tc.tile_pool(name="hpsum", bufs=2, space="PSUM") as hpsum,
            tc.tile_pool(name="mpsum", bufs=2, space="PSUM") as mpsum,
        ):
            for _rep in range(repeats):
                # ---- weights into SBUF: two blob DMAs ----
                # wqkv_sb cols: d-chunk c at 324c: [wq 0:128 | wk 128:256 | wv 256:324]
                wqkv_sb = consts.tile([128, 656], BF16, tag="wqkv_sb", name="wqkv_sb")
                nc.sync.dma_start(out=wqkv_sb, in_=wqkv[:, :], transpose=True)
                ones17 = consts.tile([1, G], BF16, tag="ones17", name="ones17")
                nc.vector.memset(ones17, 1.0)

                # ---- transposed activations via DMA transpose ----
                qt0 = consts.tile([128, R], BF16, tag="qt0", name="qt0")
                qt1 = consts.tile([128, R], BF16, tag="qt1", name="qt1")
                kt0 = consts.tile([128, N], BF16, tag="kt0", name="kt0")
                kt1 = consts.tile([128, N], BF16, tag="kt1", name="kt1")
                nc.sync.dma_start(out=qt0, in_=q[:, 0:128], transpose=True)
                nc.sync.dma_start(out=qt1, in_=q[:, 128:256], transpose=True)
                for j in range(0, N // 512):
                    sl = slice(512 * j, 512 * (j + 1))
                    nc.sync.dma_start(out=kt0[:, sl], in_=kv[sl, 0:128], transpose=True)
                    nc.sync.dma_start(out=kt1[:, sl], in_=kv[sl, 128:256], transpose=True)
                wo_all = consts.tile([G, H * D], BF16, tag="wo_all", name="wo_all")
                nc.sync.dma_start(out=wo_all, in_=wo[:, :])

                # ---- QT tiles: A = heads 0,1 (bases 0,32), B = heads 2,3 ----
                qt_sb = [consts.tile([64, R], BF16, tag=f"qt_sb{t}", name=f"qt_sb{t}")
                         for t in range(2)]
                qt_psum = mpsum.tile([128, R], F32, tag="m", name="m")
                nc.tensor.matmul(qt_psum[:], wqkv_sb[:, 0:128], qt0[:], start=True, stop=False)
                nc.tensor.matmul(qt_psum[:], wqkv_sb[:, 324:452], qt1[:], start=False, stop=True)
                nc.vector.tensor_copy(qt_sb[0][:], qt_psum[0:64, :])
                nc.vector.tensor_copy(qt_sb[1][:], qt_psum[64:128, :])

                # ---- KhT tiles + V_aug, interleaved in consumption order ----
                kht = [consts.tile([64, N], BF16, tag=f"kht{t}", name=f"kht{t}") for t in range(2)]
                v_aug = consts.tile([128, 32 * H * G], BF16, tag="v_aug", name="v_aug")
                # ones columns (pos 0 within each 17-col head group)
                v_ones = v_aug[:].rearrange("p (i g s) -> p i g s", g=H, s=G)[:, :, :, 0:1]
                nc.vector.memset(v_ones, 1.0)
                for j in range(N // 512):
                    kh_psum = mpsum.tile([128, 512], F32, tag="m", name="m")
                    nc.tensor.matmul(kh_psum[:], wqkv_sb[:, 128:256],
                                     kt0[:, 512 * j:512 * (j + 1)], start=True, stop=False)
                    nc.tensor.matmul(kh_psum[:], wqkv_sb[:, 452:580],
                                     kt1[:, 512 * j:512 * (j + 1)], start=False, stop=True)
                    nc.vector.tensor_copy(kht[0][:, 512 * j:512 * (j + 1)], kh_psum[0:64, :])
                    nc.vector.tensor_copy(kht[1][:, 512 * j:512 * (j + 1)], kh_psum[64:128, :])
                    for i in range(4 * j, 4 * j + 4):
                        v_psum = mpsum.tile([128, H * G], F32, tag="m", name="m")
                        nc.tensor.matmul(v_psum[:], kt0[:, 128 * i:128 * (i + 1)],
                                         wqkv_sb[:, 256:324], start=True, stop=False)
                        nc.tensor.matmul(v_psum[:], kt1[:, 128 * i:128 * (i + 1)],
                                         wqkv_sb[:, 580:648], start=False, stop=True)
                        # copy only the 16 V columns of each head group (skip ones col)
                        vsrc = v_psum[:].rearrange("p (g s) -> p g s", s=G)[:, :, 1:G]
                        vdst = v_aug[:, 68 * i:68 * (i + 1)].rearrange(
                            "p (g s) -> p g s", s=G)[:, :, 1:G]
                        nc.vector.tensor_copy(vdst, vsrc)

                # ---- attention, per head, software-pipelined (PE 1 pair ahead of ACT) ----
                # o_acc accumulates W_o partial products across heads (fp32 in SBUF)
                o_acc = consts.tile([128, 4 * D], F32, tag="o_acc", name="o_acc")
                NPAIR = N // 256  # 16 pairs of 128-key chunks
                for h in range(H):
                    t, b = h // 2, 32 * (h % 2)
                    kht_t, qt_t = kht[t], qt_sb[t]
                    heads_psum = hpsum.tile([G, R], F32, tag="heads", name="heads")
                    es_tiles = {}

                    def s_stage(p, kht_t=kht_t, qt_t=qt_t, b=b, es_tiles=es_tiles):
                        s_psum = spsum.tile([128, 1024], F32, tag="s", name="s")
                        lo, hi = 256 * p, 256 * p + 128
                        nc.tensor.matmul(s_psum[:, 0:512],
                                         kht_t[b:b + 16, lo:lo + 128],
                                         qt_t[b:b + 16, :], start=True, stop=True)
                        nc.tensor.matmul(s_psum[:, 512:1024],
                                         kht_t[b:b + 16, hi:hi + 128],
                                         qt_t[b:b + 16, :], start=True, stop=True)
                        es = espool.tile([128, 1024], BF16, tag="es", name="es")
                        nc.scalar.activation(es[:], s_psum[:], EXPF, scale=0.25)
                        es_tiles[p] = es

                    def av_stage(p, heads_psum=heads_psum, es_tiles=es_tiles, h=h):
                        es = es_tiles.pop(p)
                        c0, c1 = 2 * p, 2 * p + 1
                        nc.tensor.matmul(heads_psum[:],
                                         v_aug[:, 68 * c0 + 17 * h:68 * c0 + 17 * h + 17],
                                         es[:, 0:512], start=(p == 0), stop=False)
                        nc.tensor.matmul(heads_psum[:],
                                         v_aug[:, 68 * c1 + 17 * h:68 * c1 + 17 * h + 17],
                                         es[:, 512:1024], start=False, stop=(p == NPAIR - 1))

                    for p in range(NPAIR + 1):
                        if p < NPAIR:
                            s_stage(p)
                        if p >= 1:
                            av_stage(p - 1)

                    # normalize: all 17 rows scaled by 1/row0 (denominator row)
                    recip = sbops.tile([1, R], F32, tag="recip", name="recip")
                    nc.vector.reciprocal(recip[:], heads_psum[0:1, :])
                    recipb = sbops.tile([1, R], BF16, tag="recipb", name="recipb")
                    nc.vector.tensor_copy(recipb[:], recip[:])
                    rb_psum = mpsum.tile([G, R], F32, tag="m", name="m")
                    nc.tensor.matmul(rb_psum[:], ones17[:], recipb[:], start=True, stop=True)
                    headsT_sb = sbops.tile([G, R], F32, tag="headsT_sb", name="headsT_sb")
                    nc.vector.tensor_copy(headsT_sb[:], heads_psum[:])
                    # per-chunk: normalize -> W_o partial -> accumulate -> (last head) store
                    for c in range(R // 128):
                        cs = slice(128 * c, 128 * (c + 1))
                        hn_c = sbops.tile([G, 128], BF16, tag="hn", name="hn")
                        nc.vector.tensor_mul(hn_c[:], headsT_sb[:, cs], rb_psum[:, cs])
                        o_psum = mpsum.tile([128, D], F32, tag="m", name="m")
                        nc.tensor.matmul(o_psum[:], hn_c[:],
                                         wo_all[:, D * h:D * (h + 1)], start=True, stop=True)
                        osl = o_acc[:, D * c:D * (c + 1)]
                        if h == 0:
                            nc.vector.tensor_copy(osl, o_psum[:])
                        else:
                            nc.vector.tensor_add(osl, osl, o_psum[:])
                        if h == H - 1:
                            nc.sync.dma_start(out=out[cs, :], in_=osl)

    nc.finalize()
    return nc


_NC_CACHE = None


def _host_in_maps(query, key_value, W_q, W_k, W_v, W_o):
    q_bf = np.ascontiguousarray(query.astype(BF))
    kv_bf = np.ascontiguousarray(key_value.astype(BF))
    # padded wq/wk: head h at cols 64*(h//2)+32*(h%2) .. +16
    wqkv_h = np.zeros((D, 324), dtype=BF)
    wqt = np.transpose(W_q, (1, 0, 2))  # [D, H, K]
    wkt = np.transpose(W_k, (1, 0, 2))
    wvt = np.transpose(W_v, (1, 0, 2))
    for h in range(H):
        c0 = 64 * (h // 2) + 32 * (h % 2)
        wqkv_h[:, c0:c0 + K] = wqt[:, h, :].astype(BF)
        wqkv_h[:, 128 + c0:128 + c0 + K] = wkt[:, h, :].astype(BF)
        wqkv_h[:, 256 + G * h + 1:256 + G * (h + 1)] = wvt[:, h, :].astype(BF)
    wqkv_h = np.ascontiguousarray(np.concatenate(
        [wqkv_h[0:128].T, wqkv_h[128:256].T, np.zeros((8, 128), dtype=BF)], axis=0))
    wo_h = np.zeros((G, H * D), dtype=BF)
    wo_r = W_o.reshape(H, K, D)
    for h in range(H):
        wo_h[1:G, D * h:D * (h + 1)] = wo_r[h].astype(BF)
    return [{"q": q_bf[c * R:(c + 1) * R], "kv": kv_bf, "wqkv": wqkv_h, "wo": wo_h}
            for c in range(NCORES)]


def kernel(query, key_value, W_q, W_k, W_v, W_o):
    global _NC_CACHE, LAST_RESULTS
    if _NC_CACHE is None:
        _NC_CACHE = _build()
    nc = _NC_CACHE
    in_maps = _host_in_maps(query, key_value, W_q, W_k, W_v, W_o)
    res = run_bass_kernel_spmd(nc, in_maps, list(range(NCORES)), trace=TRACE)
    LAST_RESULTS = res
    return np.concatenate([res.results[c]["out"] for c in range(NCORES)], axis=0)



# revision 7
# speedup vs baseline: 3.7469x; 3.7469x over previous
"""Multi-head cross-attention TRN2 kernel (v2).

N=4096, D=256, H=4, K=16. Data-parallel over 8 NeuronCores: each core owns
512 query rows; key_value + weights replicated. No collectives.

Design notes (per core, R=512 query rows):
  - S^T form: scores chunk S^T[128 keys, 512 q] per (head, key-chunk).
  - Scores on PE in 64x128 row-tiled mode: heads packed 2 per 64-partition
    group (head h k-rows at partitions 32h..32h+16). One LDW of
    khT[64g:64g+64, chunk] serves both heads of group g; the two groups run
    concurrently on row tiles (0,0)/(64,0). Head h is selected by a
    zero-padded query operand (qt_A has heads 0,2; qt_B has heads 1,3).
  - PSUM: 3-slot ring of [128,1024] (2 banks each; slot = 2 heads x 512q)
    + 1 bank AV accumulator + 1 bank util (rb / tail).
  - Evacuation+exp of the 8.4M-element score tensor is the wall. It is
    split between ScalarE (native Exp activation) and VectorE (Schraudolph
    exp: tensor_scalar fp32->int16 (s*a+b) whose int16 bits are the bf16
    representation of ~exp(s); bitcast to bf16). Assignment alternates by
    chunk parity so each (head, query) row gets a 50/50 key mix of
    exact/approx exp (keeps the approx error averaged down).
  - AV on PE in 128x32 col-tiled mode: 4 heads concurrent, accumulating
    into one PSUM bank at partition bases 32h (rows 32h=denominator from
    the ones column, 32h+1..17 = the 16 V dims). A zeroing matmul opens the
    accumulation group so pad rows are 0.
  - Tail: strided-partition reciprocal of the 4 denominator rows, recip
    broadcast via indicator matmul, normalize, W_o matmul, DMA out.

Everything on the PE is bf16 (host-cast); accumulation fp32; output fp32.
"""
import numpy as np
import ml_dtypes

import concourse.bass as bass
from concourse import bacc
import concourse.mybir as mybir
import concourse.tile as tile
from concourse.bass_utils import run_bass_kernel_spmd

N, D, H, K = 4096, 256, 4, 16
NCORES = 8
R = N // NCORES          # 512 query rows per core
G = K + 1                # 17: ones column + 16 V dims per head
CH = 128                 # keys per chunk
NCH = N // CH            # 32 chunks
F32 = mybir.dt.float32
BF16 = mybir.dt.bfloat16
I16 = mybir.dt.int16
EXPF = mybir.ActivationFunctionType.Exp
BF = ml_dtypes.bfloat16

# Schraudolph exp for bf16 bits: exp(x) ~= bitcast_bf16(int16(x*SA + SB))
# bf16 exponent at bit 7; exp(x)=2^(x*log2e); geometric centering of the
# 2^f vs (1+f) mantissa error band (max ratio 1.0861) -> -7.62 bias.
# Scores need exp(S*0.25): fold 0.25 into the scale.
SCH_SCALE = 0.25 * 1.4426950408889634 * 128.0
SCH_BIAS = 127.0 * 128.0 - 7.62

TRACE = False
LAST_RESULTS = None

# weight blob geometry: per d-chunk dc (2 chunks of 128 d-rows):
#   cols 452*dc+0:128    wq_A (heads 0,2 at cols 32h)
#   cols 452*dc+128:256  wq_B (heads 1,3 at cols 32h)
#   cols 452*dc+256:384  wk   (head h at cols 32h)
#   cols 452*dc+384:452  wv   (head h at cols 17h+1..17h+17; col 17h zero)
WBLOB = 456  # 452 used + 4 pad (DMA transpose needs rows % 16 == 0)


def _build(repeats=1, dve_share=True):
    nc = bacc.Bacc()
    q = nc.declare_dram_parameter("q", [R, D], BF16, isOutput=False)
    kv = nc.declare_dram_parameter("kv", [N, D], BF16, isOutput=False)
    wqkv = nc.declare_dram_parameter("wqkv", [2 * WBLOB, 128], BF16, isOutput=False)
    wo = nc.declare_dram_parameter("wo", [128, D], BF16, isOutput=False)
    sel_d = nc.declare_dram_parameter("sel", [128, 128], BF16, isOutput=False)
    out = nc.declare_dram_parameter("out", [R, D], F32, isOutput=True)

    with tile.TileContext(nc) as tc:
        with (
            tc.tile_pool(name="consts", bufs=1) as consts,
            tc.tile_pool(name="esa", bufs=3) as esapool,
            tc.tile_pool(name="esd", bufs=3) as esdpool,
            tc.tile_pool(name="sbops", bufs=2) as sbops,
            tc.tile_pool(name="sring", bufs=3, space="PSUM") as sring,
            tc.tile_pool(name="accp", bufs=1, space="PSUM") as accp,
            tc.tile_pool(name="utilp", bufs=1, space="PSUM") as utilp,
        ):
            for _rep in range(repeats):
                # ---- weights into SBUF ----
                wqkv_sb = consts.tile([128, 2 * WBLOB], BF16, tag="wqkv_sb",
                                      name="wqkv_sb")
                nc.sync.dma_start(out=wqkv_sb, in_=wqkv[:, :], transpose=True)
                wo_sb = consts.tile([128, D], BF16, tag="wo_sb", name="wo_sb")
                nc.sync.dma_start(out=wo_sb, in_=wo[:, :])

                def wqA(dc):
                    return wqkv_sb[:, WBLOB * dc:WBLOB * dc + 128]

                def wqB(dc):
                    return wqkv_sb[:, WBLOB * dc + 128:WBLOB * dc + 256]

                def wk(dc):
                    return wqkv_sb[:, WBLOB * dc + 256:WBLOB * dc + 384]

                def wv(dc):
                    return wqkv_sb[:, WBLOB * dc + 384:WBLOB * dc + 452]

                # ---- transposed activations via DMA transpose ----
                qtr = [consts.tile([128, R], BF16, tag=f"qtr{i}", name=f"qtr{i}")
                       for i in range(2)]
                nc.sync.dma_start(out=qtr[0], in_=q[:, 0:128], transpose=True)
                nc.sync.dma_start(out=qtr[1], in_=q[:, 128:256], transpose=True)
                ktr = [consts.tile([128, N], BF16, tag=f"ktr{i}", name=f"ktr{i}")
                       for i in range(2)]
                for j in range(N // 512):
                    sl = slice(512 * j, 512 * (j + 1))
                    nc.sync.dma_start(out=ktr[0][:, sl], in_=kv[sl, 0:128],
                                      transpose=True)
                    nc.sync.dma_start(out=ktr[1][:, sl], in_=kv[sl, 128:256],
                                      transpose=True)

                # constant zero operands for the acc-zeroing matmul
                zcol = consts.tile([64, 128], BF16, tag="zcol", name="zcol")
                nc.vector.memset(zcol, 0.0)
                zrow = consts.tile([64, R], BF16, tag="zrow", name="zrow")
                nc.vector.memset(zrow, 0.0)
                # selector for recip broadcast: sel[32h, 32h:32h+32] = 1
                sel = consts.tile([128, 128], BF16, tag="sel", name="sel")
                nc.sync.dma_start(out=sel, in_=sel_d[:, :])
                # recip landing pad: rows 32h hold 1/denom_h, rest stay 0
                recipb = consts.tile([128, R], BF16, tag="recipb",
                                     name="recipb")
                nc.vector.memset(recipb, 0.0)

                # ---- projections ----
                # qt_A (heads 0,2) | qt_B (heads 1,3), each [128, 512]
                qp = sring.tile([128, 1024], F32, tag="s", name="s")
                for dc in range(2):
                    nc.tensor.matmul(qp[:, 0:512], wqA(dc), qtr[dc][:, :],
                                     start=(dc == 0), stop=(dc == 1))
                    nc.tensor.matmul(qp[:, 512:1024], wqB(dc), qtr[dc][:, :],
                                     start=(dc == 0), stop=(dc == 1))
                qt_sb = consts.tile([128, 1024], BF16, tag="qt_sb", name="qt_sb")
                nc.scalar.copy(qt_sb[:], qp[:])
                qtA, qtB = qt_sb[:, 0:512], qt_sb[:, 512:1024]

                # khT [128 (head h k-rows at 32h), N] bf16
                khT = consts.tile([128, N], BF16, tag="khT", name="khT")
                for c4 in range(4):
                    kp = sring.tile([128, 1024], F32, tag="s", name="s")
                    for kb in range(2):
                        for dc in range(2):
                            ks = slice(1024 * c4 + 512 * kb,
                                       1024 * c4 + 512 * kb + 512)
                            nc.tensor.matmul(kp[:, 512 * kb:512 * kb + 512],
                                             wk(dc), ktr[dc][:, ks],
                                             start=(dc == 0), stop=(dc == 1))
                    dst = khT[:, 1024 * c4:1024 * (c4 + 1)]
                    if c4 % 2 == 0:
                        nc.scalar.copy(dst, kp[:])
                    else:
                        nc.vector.tensor_copy(dst, kp[:])

                # v_aug [128, 32*68]: chunk c cols 68c..68c+68; within: head h
                # at 17h (ones col) + 1..17 (V dims)
                v_aug = consts.tile([128, NCH * 4 * G], BF16, tag="v_aug",
                                    name="v_aug")
                v_ones = v_aug[:].rearrange("p (c h s) -> p c h s", h=H,
                                            s=G)[:, :, :, 0:1]
                nc.vector.memset(v_ones, 1.0)
                for vb in range(8):  # 4 chunks per slot
                    vp = sring.tile([128, 1024], F32, tag="s", name="s")
                    for j in range(4):
                        ck = 4 * vb + j
                        for dc in range(2):
                            nc.tensor.matmul(
                                vp[:, 68 * j:68 * (j + 1)],
                                ktr[dc][:, 128 * ck:128 * (ck + 1)],
                                wv(dc), start=(dc == 0), stop=(dc == 1))
                    vsrc = vp[:, 0:272].rearrange("p (j h s) -> p j h s",
                                                  h=H, s=G)[:, :, :, 1:G]
                    vdst = v_aug[:, 68 * 4 * vb:68 * 4 * (vb + 1)].rearrange(
                        "p (j h s) -> p j h s", h=H, s=G)[:, :, :, 1:G]
                    if vb % 2 == 0:
                        nc.vector.tensor_copy(vdst, vsrc)
                    else:
                        nc.scalar.copy(vdst, vsrc)

                # ---- main loop ----
                acc = accp.tile([128, R], F32, tag="acc", name="acc")
                # zero the acc bank (pad rows must be 0; opens the accum group)
                nc.tensor.matmul(acc[:], zcol[:], zrow[:], start=True,
                                 stop=False, skip_group_check=True)

                es_of = {}  # chunk -> (tile, col_base_is_h01)

                def scores(c):
                    lo = sring.tile([128, 1024], F32, tag="s", name="s")  # h0,h1
                    hi = sring.tile([128, 1024], F32, tag="s", name="s")  # h2,h3
                    for g in range(2):
                        ksl = khT[64 * g:64 * g + 64, 128 * c:128 * (c + 1)]
                        slot = lo if g == 0 else hi
                        nc.tensor.matmul(slot[:, 0:512], ksl,
                                         qtA[64 * g:64 * g + 64, :],
                                         start=True, stop=True)
                        nc.tensor.matmul(slot[:, 512:1024], ksl,
                                         qtB[64 * g:64 * g + 64, :],
                                         start=True, stop=True)
                    # evacuate + exp; alternate engines by parity
                    act_slot, dve_slot = (lo, hi) if c % 2 == 0 else (hi, lo)
                    es_act = esapool.tile([128, 1024], BF16, tag="esa",
                                          name="esa")
                    nc.scalar.activation(es_act[:], act_slot[:], EXPF,
                                         scale=0.25)
                    if dve_share:
                        es_i16 = esdpool.tile([128, 1024], I16, tag="esd",
                                              name="esd")
                        nc.vector.tensor_scalar(
                            es_i16[:], dve_slot[:], SCH_SCALE, SCH_BIAS,
                            mybir.AluOpType.mult, mybir.AluOpType.add)
                        es_dve = es_i16.bitcast(BF16)
                    else:
                        es_dve = esdpool.tile([128, 1024], BF16, tag="esd",
                                              name="esd")
                        nc.vector.tensor_scalar(
                            es_dve[:], dve_slot[:], 1.0, None,
                            mybir.AluOpType.mult)
                        # fallback: no exp on DVE (debug only)
                    es_of[c] = (es_act, es_dve, c % 2 == 0)

                def av(c):
                    es_act, es_dve, act_is_h01 = es_of.pop(c)
                    for h in range(H):
                        src = es_act if (h < 2) == act_is_h01 else es_dve
                        esl = src[:, 512 * (h % 2):512 * (h % 2) + 512]
                        nc.tensor.matmul(
                            acc[32 * h:32 * h + G, :],
                            v_aug[:, 68 * c + G * h:68 * c + G * (h + 1)],
                            esl, start=False, stop=(c == NCH - 1 and h == H - 1),
                            tile_position=(0, 32 * h), skip_group_check=True)

                BATCH = 4
                for b in range(NCH // BATCH):
                    for c in range(BATCH * b, BATCH * (b + 1)):
                        scores(c)
                    if b > 0:
                        for c in range(BATCH * (b - 1), BATCH * b):
                            av(c)
                for c in range(NCH - BATCH, NCH):
                    av(c)

                # ---- tail: normalize + W_o ----
                with nc.allow_low_precision(reason="bf16 recip feeds bf16 mm"):
                    for h in range(H):
                        nc.vector.reciprocal(recipb[32 * h:32 * h + 1, :],
                                             acc[32 * h:32 * h + 1, :])
                rb = utilp.tile([128, R], F32, tag="rb", name="rb")
                nc.tensor.matmul(rb[:], sel[:], recipb[:], start=True,
                                 stop=True)
                acc_sb = sbops.tile([128, R], F32, tag="acc_sb", name="acc_sb")
                nc.scalar.copy(acc_sb[:], acc[:])
                hn = sbops.tile([128, R], BF16, tag="hn", name="hn")
                nc.vector.tensor_mul(hn[:], acc_sb[:], rb[:])
                wop = sring.tile([128, 1024], F32, tag="s", name="s")
                for qc in range(4):
                    nc.tensor.matmul(wop[:, 256 * qc:256 * (qc + 1)],
                                     hn[:, 128 * qc:128 * (qc + 1)],
                                     wo_sb[:, :], start=True, stop=True)
                out_sb = sbops.tile([128, 1024], F32, tag="out_sb",
                                    name="out_sb")
                nc.scalar.copy(out_sb[:, 0:512], wop[:, 0:512])
                nc.vector.tensor_copy(out_sb[:, 512:1024], wop[:, 512:1024])
                for qc in range(4):
                    nc.sync.dma_start(
                        out=out[128 * qc:128 * (qc + 1), :],
                        in_=out_sb[:, 256 * qc:256 * (qc + 1)])

    nc.finalize()
    return nc


_NC_CACHE = None


def _host_in_maps(query, key_value, W_q, W_k, W_v, W_o):
    q_bf = np.ascontiguousarray(query.astype(BF))
    kv_bf = np.ascontiguousarray(key_value.astype(BF))
    # blob [2*WBLOB, 128]: row 452*dc + c = column c of the per-d-chunk
    # weight block (see WBLOB comment); DMA transpose puts it at
    # wqkv_sb[:, 452*dc + c].
    wqt = np.transpose(W_q, (1, 0, 2))  # [D, H, K]
    wkt = np.transpose(W_k, (1, 0, 2))
    wvt = np.transpose(W_v, (1, 0, 2))
    blk = np.zeros((D, WBLOB), dtype=np.float32)
    for h in range(H):
        cq = 32 * h
        dst = 0 if h % 2 == 0 else 128  # wq_A vs wq_B
        blk[:, dst + cq:dst + cq + K] = wqt[:, h, :]
        blk[:, 256 + cq:256 + cq + K] = wkt[:, h, :]
        blk[:, 384 + G * h + 1:384 + G * (h + 1)] = wvt[:, h, :]
    blob = np.concatenate([blk[0:128].T, blk[128:256].T], axis=0).astype(BF)
    blob = np.ascontiguousarray(blob)
    # wo blob [128, D]: row 32h+1+k = W_o[16h+k, :]; other rows zero
    wo_h = np.zeros((128, D), dtype=BF)
    wo_r = W_o.reshape(H, K, D)
    for h in range(H):
        wo_h[32 * h + 1:32 * h + 1 + K, :] = wo_r[h].astype(BF)
    sel = np.zeros((128, 128), dtype=BF)
    for h in range(H):
        sel[32 * h, 32 * h:32 * h + 32] = 1.0
    return [{"q": q_bf[c * R:(c + 1) * R], "kv": kv_bf, "wqkv": blob,
             "wo": wo_h, "sel": sel} for c in range(NCORES)]


def kernel(query, key_value, W_q, W_k, W_v, W_o):
    global _NC_CACHE, LAST_RESULTS
    if _NC_CACHE is None:
        _NC_CACHE = _build()
    nc = _NC_CACHE
    in_maps = _host_in_maps(query, key_value, W_q, W_k, W_v, W_o)
    res = run_bass_kernel_spmd(nc, in_maps, list(range(NCORES)), trace=TRACE)
    LAST_RESULTS = res
    return np.concatenate([res.results[c]["out"] for c in range(NCORES)], axis=0)


# revision 9
# speedup vs baseline: 4.8102x; 1.2838x over previous
"""Multi-head cross-attention TRN2 kernel (v2).

N=4096, D=256, H=4, K=16. Data-parallel over 8 NeuronCores: each core owns
512 query rows; key_value + weights replicated. No collectives.

Design notes (per core, R=512 query rows):
  - S^T form: scores chunk S^T[128 keys, 512 q] per (head, key-chunk).
  - Scores on PE in 64x128 row-tiled mode: heads packed 2 per 64-partition
    group (head h k-rows at partitions 32h..32h+16). One LDW of
    khT[64g:64g+64, chunk] serves both heads of group g; the two groups run
    concurrently on row tiles (0,0)/(64,0). Head h is selected by a
    zero-padded query operand (qt_A has heads 0,2; qt_B has heads 1,3).
  - PSUM: 3-slot ring of [128,1024] (2 banks each; slot = 2 heads x 512q)
    + 1 bank AV accumulator + 1 bank util (rb / tail).
  - Evacuation+exp of the 8.4M-element score tensor is the wall. It is
    split between ScalarE (native Exp activation) and VectorE (Schraudolph
    exp: tensor_scalar fp32->int16 (s*a+b) whose int16 bits are the bf16
    representation of ~exp(s); bitcast to bf16). Assignment alternates by
    chunk parity so each (head, query) row gets a 50/50 key mix of
    exact/approx exp (keeps the approx error averaged down).
  - AV on PE in 128x32 col-tiled mode: 4 heads concurrent, accumulating
    into one PSUM bank at partition bases 32h (rows 32h=denominator from
    the ones column, 32h+1..17 = the 16 V dims). A zeroing matmul opens the
    accumulation group so pad rows are 0.
  - Tail: strided-partition reciprocal of the 4 denominator rows, recip
    broadcast via indicator matmul, normalize, W_o matmul, DMA out.

Everything on the PE is bf16 (host-cast); accumulation fp32; output fp32.
"""
import numpy as np
import ml_dtypes

import concourse.bass as bass
from concourse import bacc
import concourse.mybir as mybir
import concourse.tile as tile
from concourse.bass_utils import run_bass_kernel_spmd

N, D, H, K = 4096, 256, 4, 16
NCORES = 8
R = N // NCORES          # 512 query rows per core
G = K + 1                # 17: ones column + 16 V dims per head
CH = 128                 # keys per chunk
NCH = N // CH            # 32 chunks
F32 = mybir.dt.float32
BF16 = mybir.dt.bfloat16
I16 = mybir.dt.int16
EXPF = mybir.ActivationFunctionType.Exp
BF = ml_dtypes.bfloat16

# Schraudolph exp for bf16 bits: exp(x) ~= bitcast_bf16(int16(x*SA + SB))
# bf16 exponent at bit 7; exp(x)=2^(x*log2e); geometric centering of the
# 2^f vs (1+f) mantissa error band (max ratio 1.0861) -> -7.62 bias.
# Scores need exp(S*0.25): fold 0.25 into the scale.
SCH_SCALE = 0.25 * 1.4426950408889634 * 128.0
SCH_BIAS = 127.0 * 128.0 - 7.62

TRACE = False
LAST_RESULTS = None

# weight blob geometry: per d-chunk dc (2 chunks of 128 d-rows):
#   cols 452*dc+0:128    wq_A (heads 0,2 at cols 32h)
#   cols 452*dc+128:256  wq_B (heads 1,3 at cols 32h)
#   cols 452*dc+256:384  wk   (head h at cols 32h)
#   cols 452*dc+384:452  wv   (head h at cols 17h+1..17h+17; col 17h zero)
WBLOB = 456  # 452 used + 4 pad (DMA transpose needs rows % 16 == 0)


def _build(repeats=1, dve_share=True):
    nc = bacc.Bacc()
    q = nc.declare_dram_parameter("q", [R, D], BF16, isOutput=False)
    kv = nc.declare_dram_parameter("kv", [N, D], BF16, isOutput=False)
    wqkv = nc.declare_dram_parameter("wqkv", [2 * WBLOB, 128], BF16, isOutput=False)
    wo = nc.declare_dram_parameter("wo", [128, D], BF16, isOutput=False)
    sel_d = nc.declare_dram_parameter("sel", [128, 128], F32, isOutput=False)
    out = nc.declare_dram_parameter("out", [R, D], F32, isOutput=True)

    with tile.TileContext(nc) as tc:
        with (
            tc.tile_pool(name="consts", bufs=1) as consts,
            tc.tile_pool(name="esa", bufs=3) as esapool,
            tc.tile_pool(name="esd", bufs=3) as esdpool,
            tc.tile_pool(name="sbops", bufs=2) as sbops,
            tc.tile_pool(name="sring", bufs=3, space="PSUM") as sring,
            tc.tile_pool(name="accp", bufs=1, space="PSUM") as accp,
            tc.tile_pool(name="utilp", bufs=1, space="PSUM") as utilp,
        ):
            for _rep in range(repeats):
                # ---- weights into SBUF ----
                wqkv_sb = consts.tile([128, 2 * WBLOB], BF16, tag="wqkv_sb",
                                      name="wqkv_sb")
                nc.sync.dma_start(out=wqkv_sb, in_=wqkv[:, :], transpose=True)
                wo_sb = consts.tile([128, D], BF16, tag="wo_sb", name="wo_sb")
                nc.sync.dma_start(out=wo_sb, in_=wo[:, :])

                def wqA(dc):
                    return wqkv_sb[:, WBLOB * dc:WBLOB * dc + 128]

                def wqB(dc):
                    return wqkv_sb[:, WBLOB * dc + 128:WBLOB * dc + 256]

                def wk(dc):
                    return wqkv_sb[:, WBLOB * dc + 256:WBLOB * dc + 384]

                def wv(dc):
                    return wqkv_sb[:, WBLOB * dc + 384:WBLOB * dc + 452]

                # ---- transposed activations via DMA transpose ----
                qtr = [consts.tile([128, R], BF16, tag=f"qtr{i}", name=f"qtr{i}")
                       for i in range(2)]
                nc.sync.dma_start(out=qtr[0], in_=q[:, 0:128], transpose=True)
                nc.sync.dma_start(out=qtr[1], in_=q[:, 128:256], transpose=True)
                ktr = [consts.tile([128, N], BF16, tag=f"ktr{i}", name=f"ktr{i}")
                       for i in range(2)]
                for j in range(N // 512):
                    sl = slice(512 * j, 512 * (j + 1))
                    nc.sync.dma_start(out=ktr[0][:, sl], in_=kv[sl, 0:128],
                                      transpose=True)
                    nc.sync.dma_start(out=ktr[1][:, sl], in_=kv[sl, 128:256],
                                      transpose=True)

                # constant zero operands for the acc-zeroing matmul
                zcol = consts.tile([64, 128], BF16, tag="zcol", name="zcol")
                nc.vector.memset(zcol, 0.0)
                zrow = consts.tile([64, R], BF16, tag="zrow", name="zrow")
                nc.vector.memset(zrow, 0.0)
                # selector for denom broadcast: sel[32h, 32h:32h+32] = 1
                sel = consts.tile([128, 128], F32, tag="sel", name="sel")
                nc.sync.dma_start(out=sel, in_=sel_d[:, :])

                # ---- projections ----
                # qt_A (heads 0,2) | qt_B (heads 1,3), each [128, 512]
                qp = sring.tile([128, 1024], F32, tag="s", name="s")
                for dc in range(2):
                    nc.tensor.matmul(qp[:, 0:512], wqA(dc), qtr[dc][:, :],
                                     start=(dc == 0), stop=(dc == 1))
                    nc.tensor.matmul(qp[:, 512:1024], wqB(dc), qtr[dc][:, :],
                                     start=(dc == 0), stop=(dc == 1))
                qt_sb = consts.tile([128, 1024], BF16, tag="qt_sb", name="qt_sb")
                nc.scalar.copy(qt_sb[:], qp[:])
                qtA, qtB = qt_sb[:, 0:512], qt_sb[:, 512:1024]

                # khT [128 (head h k-rows at 32h), N] bf16
                khT = consts.tile([128, N], BF16, tag="khT", name="khT")
                for c4 in range(4):
                    kp = sring.tile([128, 1024], F32, tag="s", name="s")
                    for kb in range(2):
                        for dc in range(2):
                            ks = slice(1024 * c4 + 512 * kb,
                                       1024 * c4 + 512 * kb + 512)
                            nc.tensor.matmul(kp[:, 512 * kb:512 * kb + 512],
                                             wk(dc), ktr[dc][:, ks],
                                             start=(dc == 0), stop=(dc == 1))
                    dst = khT[:, 1024 * c4:1024 * (c4 + 1)]
                    if c4 == 3:
                        nc.vector.tensor_copy(dst, kp[:])
                    else:
                        nc.scalar.copy(dst, kp[:])

                # v_aug [128, 32*68]: chunk c cols 68c..68c+68; within: head h
                # at 17h (ones col) + 1..17 (V dims)
                v_aug = consts.tile([128, NCH * 4 * G], BF16, tag="v_aug",
                                    name="v_aug")
                v_ones = v_aug[:].rearrange("p (c h s) -> p c h s", h=H,
                                            s=G)[:, :, :, 0:1]
                nc.vector.memset(v_ones, 1.0)
                for vb in range(8):  # 4 chunks per slot
                    vp = sring.tile([128, 1024], F32, tag="s", name="s")
                    for j in range(4):
                        ck = 4 * vb + j
                        for dc in range(2):
                            nc.tensor.matmul(
                                vp[:, 68 * j:68 * (j + 1)],
                                ktr[dc][:, 128 * ck:128 * (ck + 1)],
                                wv(dc), start=(dc == 0), stop=(dc == 1))
                    vsrc = vp[:, 0:272].rearrange("p (j h s) -> p j h s",
                                                  h=H, s=G)[:, :, :, 1:G]
                    vdst = v_aug[:, 68 * 4 * vb:68 * 4 * (vb + 1)].rearrange(
                        "p (j h s) -> p j h s", h=H, s=G)[:, :, :, 1:G]
                    if vb % 2 == 0:
                        nc.vector.tensor_copy(vdst, vsrc)
                    else:
                        nc.scalar.copy(vdst, vsrc)

                # ---- main loop ----
                acc = accp.tile([128, R], F32, tag="acc", name="acc")
                # zero the acc bank (pad rows must be 0; opens the accum group)
                nc.tensor.matmul(acc[:], zcol[:], zrow[:], start=True,
                                 stop=False, skip_group_check=True)

                es_of = {}  # chunk -> (tile, col_base_is_h01)

                def scores(c):
                    lo = sring.tile([128, 1024], F32, tag="s", name="s")  # h0,h1
                    hi = sring.tile([128, 1024], F32, tag="s", name="s")  # h2,h3
                    for g in range(2):
                        ksl = khT[64 * g:64 * g + 64, 128 * c:128 * (c + 1)]
                        slot = lo if g == 0 else hi
                        nc.tensor.matmul(slot[:, 0:512], ksl,
                                         qtA[64 * g:64 * g + 64, :],
                                         start=True, stop=True)
                        nc.tensor.matmul(slot[:, 512:1024], ksl,
                                         qtB[64 * g:64 * g + 64, :],
                                         start=True, stop=True)
                    # evacuate + exp; alternate engines by parity
                    act_slot, dve_slot = (lo, hi) if c % 2 == 0 else (hi, lo)
                    es_act = esapool.tile([128, 1024], BF16, tag="esa",
                                          name="esa")
                    nc.scalar.activation(es_act[:], act_slot[:], EXPF,
                                         scale=0.25)
                    if dve_share:
                        es_i16 = esdpool.tile([128, 1024], I16, tag="esd",
                                              name="esd")
                        nc.vector.tensor_scalar(
                            es_i16[:], dve_slot[:], SCH_SCALE, SCH_BIAS,
                            mybir.AluOpType.mult, mybir.AluOpType.add)
                        es_dve = es_i16.bitcast(BF16)
                    else:
                        es_dve = esdpool.tile([128, 1024], BF16, tag="esd",
                                              name="esd")
                        nc.vector.tensor_scalar(
                            es_dve[:], dve_slot[:], 1.0, None,
                            mybir.AluOpType.mult)
                        # fallback: no exp on DVE (debug only)
                    es_of[c] = (es_act, es_dve, c % 2 == 0)

                def av(c):
                    es_act, es_dve, act_is_h01 = es_of.pop(c)
                    for h in range(H):
                        src = es_act if (h < 2) == act_is_h01 else es_dve
                        esl = src[:, 512 * (h % 2):512 * (h % 2) + 512]
                        nc.tensor.matmul(
                            acc[32 * h:32 * h + G, :],
                            v_aug[:, 68 * c + G * h:68 * c + G * (h + 1)],
                            esl, start=False, stop=(c == NCH - 1 and h == H - 1),
                            tile_position=(0, 32 * h), skip_group_check=True)

                BATCH = 8
                for b in range(NCH // BATCH):
                    for c in range(BATCH * b, BATCH * (b + 1)):
                        scores(c)
                    if b > 0:
                        for c in range(BATCH * (b - 1), BATCH * b):
                            av(c)
                for c in range(NCH - BATCH, NCH):
                    av(c)

                # ---- tail: normalize + W_o ----
                acc_sb = sbops.tile([128, R], F32, tag="acc_sb", name="acc_sb")
                nc.scalar.copy(acc_sb[:], acc[:])
                # rb = per-row broadcast of the head's denominator (fp32 mm)
                rb = utilp.tile([128, R], F32, tag="rb", name="rb")
                nc.tensor.matmul(rb[:], sel[:], acc_sb[:], start=True,
                                 stop=True)
                rbr = sbops.tile([128, R], BF16, tag="rbr", name="rbr")
                with nc.allow_low_precision(reason="bf16 recip feeds bf16 mm"):
                    nc.vector.reciprocal(rbr[:], rb[:])
                hn = sbops.tile([128, R], BF16, tag="hn", name="hn")
                nc.vector.tensor_mul(hn[:], acc_sb[:], rbr[:])
                wop = sring.tile([128, 1024], F32, tag="s", name="s")
                for qc in range(4):
                    nc.tensor.matmul(wop[:, 256 * qc:256 * (qc + 1)],
                                     hn[:, 128 * qc:128 * (qc + 1)],
                                     wo_sb[:, :], start=True, stop=True)
                out_sb = sbops.tile([128, 1024], F32, tag="out_sb",
                                    name="out_sb")
                nc.scalar.copy(out_sb[:, 0:512], wop[:, 0:512])
                nc.vector.tensor_copy(out_sb[:, 512:1024], wop[:, 512:1024])
                for qc in range(4):
                    nc.sync.dma_start(
                        out=out[128 * qc:128 * (qc + 1), :],
                        in_=out_sb[:, 256 * qc:256 * (qc + 1)])

    nc.finalize()
    return nc


_NC_CACHE = None


def _host_in_maps(query, key_value, W_q, W_k, W_v, W_o):
    q_bf = np.ascontiguousarray(query.astype(BF))
    kv_bf = np.ascontiguousarray(key_value.astype(BF))
    # blob [2*WBLOB, 128]: row 452*dc + c = column c of the per-d-chunk
    # weight block (see WBLOB comment); DMA transpose puts it at
    # wqkv_sb[:, 452*dc + c].
    wqt = np.transpose(W_q, (1, 0, 2))  # [D, H, K]
    wkt = np.transpose(W_k, (1, 0, 2))
    wvt = np.transpose(W_v, (1, 0, 2))
    blk = np.zeros((D, WBLOB), dtype=np.float32)
    for h in range(H):
        cq = 32 * h
        dst = 0 if h % 2 == 0 else 128  # wq_A vs wq_B
        blk[:, dst + cq:dst + cq + K] = wqt[:, h, :]
        blk[:, 256 + cq:256 + cq + K] = wkt[:, h, :]
        blk[:, 384 + G * h + 1:384 + G * (h + 1)] = wvt[:, h, :]
    blob = np.concatenate([blk[0:128].T, blk[128:256].T], axis=0).astype(BF)
    blob = np.ascontiguousarray(blob)
    # wo blob [128, D]: row 32h+1+k = W_o[16h+k, :]; other rows zero
    wo_h = np.zeros((128, D), dtype=BF)
    wo_r = W_o.reshape(H, K, D)
    for h in range(H):
        wo_h[32 * h + 1:32 * h + 1 + K, :] = wo_r[h].astype(BF)
    sel = np.zeros((128, 128), dtype=np.float32)
    for h in range(H):
        sel[32 * h, 32 * h:32 * h + 32] = 1.0
    return [{"q": q_bf[c * R:(c + 1) * R], "kv": kv_bf, "wqkv": blob,
             "wo": wo_h, "sel": sel} for c in range(NCORES)]


def kernel(query, key_value, W_q, W_k, W_v, W_o):
    global _NC_CACHE, LAST_RESULTS
    if _NC_CACHE is None:
        _NC_CACHE = _build()
    nc = _NC_CACHE
    in_maps = _host_in_maps(query, key_value, W_q, W_k, W_v, W_o)
    res = run_bass_kernel_spmd(nc, in_maps, list(range(NCORES)), trace=TRACE)
    LAST_RESULTS = res
    return np.concatenate([res.results[c]["out"] for c in range(NCORES)], axis=0)
